# revision 23
# baseline (speedup 1.0000x reference)
"""AdaptiveEmbedding forward on 8 TRN2 NeuronCores (Bass/Tile, SPMD).

Strategy (data-parallel, routing-as-sharding):
  - Tokens are routed host-side into 7 segments: head (vocab [0,20000)),
    2 tail0 chunks of 30000 ids, 4 tail1 chunks of 30000 ids (chunking is
    required because dma_gather indices are int16).  Each segment's global
    token list is dealt round-robin over the 8 cores, so per-core segment
    sizes are uniform (SPMD: one program, identical shapes on all cores).
  - Head cluster: the projection is folded into the table host-side
    (tproj = table0 @ proj0 * 32, f16), so on device head tokens are a
    pure dma_gather of 2 KiB rows that go straight back out to DRAM.
  - Tail0 (d=256): f16 table, transposed dma_gather lands embeddings
    directly in [d_partition, token] layout; two-k-tile f16 matmuls with
    the (pre-scaled) projection; f32 PSUM.
  - Tail1 (d=64): f32 gather in token-major layout, PE-transpose per
    128-token group, packed into one [64, n] f16 operand, K=64 matmuls.
  - Gathers fan out over all 4 SWDGE queues (parallel Q7 descriptor
    generation); PSUM->SBUF copies split across VectorE/ScalarE; outputs
    are written f16 per segment and the host inverse-permutes + upcasts
    to the [8, 1024, 1024] f32 result.
"""

import numpy as np

import concourse.bass as bass
import concourse.tile as tile
from concourse import bacc, mybir, masks
from concourse.bass_utils import run_bass_kernel_spmd

HIDDEN = 1024
N_CORES = 8
CHUNK = 30000
HEAD_V = 20000
# (lo, rows) per gather segment; chunks keep local indices < int16 max.
SEGS = (
    (0, HEAD_V),
    (20000, CHUNK), (50000, CHUNK),
    (80000, CHUNK), (110000, CHUNK), (140000, CHUNK), (170000, CHUNK),
)
N_T1 = 4  # tail1 chunk count (segments 3..6)

F32 = mybir.dt.float32

F16 = mybir.dt.float16
I16 = mybir.dt.int16

# dtype knobs: fp16 halves the dominant DMA traffic (output rows + head
# table gather) at ~2e-4 relative error; tail1 math stays f32 end-to-end.
OUT_DT, OUT_NP = F16, np.float16
T0_DT, T0_NP = F16, np.float16

# A/B knobs (env-overridable for benching; defaults are the shipped config)
import os as _os
K_QUEUES = int(_os.environ.get("K_QUEUES", "4"))
K_IDX_SPLIT = int(_os.environ.get("K_IDX_SPLIT", "1"))
K_COPY_SPLIT = int(_os.environ.get("K_COPY_SPLIT", "1"))
K_MERGE_OUT = int(_os.environ.get("K_MERGE_OUT", "1"))


def _ceil(a, m):
    return -(-a // m) * m


def _plan(x):
    """Host-side routing: per-segment per-core token lists + idx packing."""
    flat = np.asarray(x).reshape(-1).astype(np.int64)
    seg_tok = []      # global token positions per (segment, core)
    seg_nmax = []     # uniform per-core count per segment
    for lo, rows in SEGS:
        sel = np.nonzero((flat >= lo) & (flat < lo + rows))[0]
        toks = [sel[c::N_CORES] for c in range(N_CORES)]
        seg_tok.append(toks)
        seg_nmax.append(max(len(t) for t in toks))

    # idx free-dim column layout: segment s occupies cols [off16[s], +npad/16)
    npad = [max(128, _ceil(n, 128)) if n > 0 else 0 for n in seg_nmax]
    off16 = np.cumsum([0] + [p // 16 for p in npad])
    tot16 = int(off16[-1])

    idx_arrs = []
    for c in range(N_CORES):
        arr = np.zeros((16, tot16), np.int16)
        for s, (lo, rows) in enumerate(SEGS):
            if npad[s] == 0:
                continue
            ids = (flat[seg_tok[s][c]] - lo).astype(np.int16)
            i = np.arange(len(ids))
            arr[i % 16, int(off16[s]) + i // 16] = ids
        # the GPSIMD Q7 cores each read their own 16-partition stripe:
        # the wrapped index pattern must be replicated across all 8 stripes
        idx_arrs.append(np.tile(arr, (8, 1)))

    # output row bases: head+tail0 segments padded to 128-row groups,
    # tail1 chunks packed back-to-back
    row_base = [0, npad[0], npad[0] + npad[1]]
    t1_base = npad[0] + npad[1] + npad[2]
    pack_off = np.cumsum([0] + seg_nmax[3:3 + N_T1]).tolist()
    packtot = pack_off[-1]
    tot_rows = t1_base + max(_ceil(packtot, 128), 128)

    return dict(
        seg_tok=seg_tok, nmax=seg_nmax, npad=npad,
        off16=[int(v) for v in off16], tot16=tot16,
        row_base=row_base, t1_base=t1_base, pack_off=pack_off,
        packtot=packtot, tot_rows=tot_rows, idx=idx_arrs,
    )


def _emit_body(nc, tc, ctx, P, T):
    nmax, npad, off16 = P["nmax"], P["npad"], P["off16"]
    packtot = P["packtot"]

    tproj = T["tproj"].ap()
    t0b = T["t0b"].ap()
    t1 = T["t1"].ap()
    out = T["out"].ap()

    const = ctx.enter_context(tc.tile_pool(name="const", bufs=1))
    bufs = ctx.enter_context(tc.tile_pool(name="bufs", bufs=1))
    outp = ctx.enter_context(tc.tile_pool(name="outp", bufs=4))
    psum_mm = ctx.enter_context(tc.tile_pool(name="psum_mm", bufs=3, space="PSUM"))
    psum_tp = ctx.enter_context(tc.tile_pool(name="psum_tp", bufs=2, space="PSUM"))

    # constants / weights: per-segment idx slices so each gather waits
    # only for its own few-KB index DMA
    idx_all = T["idx"].ap()
    idx_tiles = {}
    if K_IDX_SPLIT:
        for s in range(len(SEGS)):
            if not npad[s]:
                continue
            it = const.tile([128, npad[s] // 16], I16, tag=f"idx{s}", name=f"idx{s}")
            nc.sync.dma_start(it[:], idx_all[:, off16[s]:off16[s + 1]])
            idx_tiles[s] = it
    else:
        idx_sb = const.tile([128, P["tot16"]], I16, tag="idx")
        nc.sync.dma_start(idx_sb[:], idx_all[:])
        for s in range(len(SEGS)):
            if npad[s]:
                idx_tiles[s] = idx_sb[:, off16[s]:off16[s + 1]]
    p1_sb = const.tile([128, 2, HIDDEN], T0_DT, tag="p1")
    nc.scalar.dma_start(p1_sb[:], T["p1"].ap()[:])
    p2_sb = const.tile([64, HIDDEN], F16, tag="p2")
    nc.scalar.dma_start(p2_sb[:], T["p2"].ap()[:])
    # ---- phase 1: issue every gather up front (overlaps everything) ----
    hbuf = None
    if npad[0]:
        hbuf = bufs.tile([128, npad[0] // 128, HIDDEN], OUT_DT, tag="head")
        nc.gpsimd.dma_gather(
            hbuf[:], tproj[:], idx_tiles[0][:, :],
            num_idxs=npad[0], num_idxs_reg=npad[0], elem_size=HIDDEN,
            queue_num=0,
        )
    e1t = [None, None]
    for ci, s in enumerate((1, 2)):
        if not npad[s]:
            continue
        e1t[ci] = bufs.tile([128, 2, npad[s]], T0_DT, tag=f"e1t{ci}", name=f"e1t{ci}")
        nc.gpsimd.dma_gather(
            e1t[ci][:], t0b[ci * CHUNK:(ci + 1) * CHUNK, :],
            idx_tiles[s][:, :],
            num_idxs=npad[s], num_idxs_reg=npad[s], elem_size=256,
            transpose=True, queue_num=(1 + ci) % K_QUEUES,
        )
    g2 = [None] * N_T1
    for ci in range(N_T1):
        s = 3 + ci
        if not npad[s]:
            continue
        g2[ci] = bufs.tile([128, npad[s] // 128, 64], F32, tag=f"g2_{ci}", name=f"g2_{ci}")
        nc.gpsimd.dma_gather(
            g2[ci][:], t1[ci * CHUNK:(ci + 1) * CHUNK, :],
            idx_tiles[s][:, :],
            num_idxs=npad[s], num_idxs_reg=npad[s], elem_size=64,
            queue_num=(3 - ci) % K_QUEUES,
        )

    ident = const.tile([128, 128], F32, tag="ident")
    masks.make_identity(nc, ident[:])

    # ---- head rows go straight out ----
    if npad[0]:
        for g in range(npad[0] // 128):
            v = min(128, nmax[0] - 128 * g)
            if v <= 0:
                break
            nc.sync.dma_start(out[128 * g:128 * g + v, :], hbuf[0:v, g, :])

    # ---- tail1 transposes early (PE) so packed matmuls can start ----
    e2t = None
    if packtot:
        e2t = bufs.tile([64, _ceil(packtot, 128)], F16, tag="e2t")
        nc.vector.memset(e2t[:], 0.0)
        for ci in range(N_T1):
            s = 3 + ci
            if not npad[s]:
                continue
            for g in range(npad[s] // 128):
                v = min(128, nmax[s] - 128 * g)
                if v <= 0:
                    break
                tp = psum_tp.tile([64, 128], F32, tag="tp")
                nc.tensor.transpose(tp[:, :], g2[ci][:, g, :], ident[:])
                off = P["pack_off"][ci] + 128 * g
                nc.vector.tensor_copy(e2t[:, off:off + v], tp[:, 0:v])

    # ---- tail0 matmuls ----
    dma_eng = [nc.sync, nc.scalar]
    for ci, s in enumerate((1, 2)):
        if not npad[s]:
            continue
        ng = npad[s] // 128
        stage = None
        if K_MERGE_OUT:
            stage = outp.tile([128, ng, HIDDEN], OUT_DT, tag=f"st0_{ci}",
                              name=f"st0_{ci}")
        for g in range(ng):
            v = 128 if K_MERGE_OUT else min(128, nmax[s] - 128 * g)
            if v <= 0:
                break
            ps = psum_mm.tile([128, HIDDEN], F32, tag="mm")
            for kt in range(2):
                for nt in range(2):
                    nc.tensor.matmul(
                        ps[0:v, nt * 512:(nt + 1) * 512],
                        e1t[ci][:, kt, 128 * g:128 * g + v],
                        p1_sb[:, kt, nt * 512:(nt + 1) * 512],
                        start=(kt == 0), stop=(kt == 1),
                    )
            ot = stage[:, g, :] if K_MERGE_OUT else outp.tile(
                [128, HIDDEN], OUT_DT, tag="ot")
            if K_COPY_SPLIT:
                nc.vector.tensor_copy(ot[0:v, 0:512], ps[0:v, 0:512])
                nc.scalar.copy(ot[0:v, 512:1024], ps[0:v, 512:1024])
            else:
                nc.vector.tensor_copy(ot[0:v, :], ps[0:v, :])
            if not K_MERGE_OUT:
                r0 = P["row_base"][s] + 128 * g
                nc.sync.dma_start(out[r0:r0 + v, :], ot[0:v, :])
        if K_MERGE_OUT:
            r0 = P["row_base"][s]
            dst = out[r0:r0 + ng * 128, :].rearrange(
                "(g p) h -> p g h", p=128)
            dma_eng[ci % 2].dma_start(dst, stage[:])

    # ---- tail1 packed matmuls ----
    if packtot:
        ngm = -(-packtot // 128)
        stage1 = None
        if K_MERGE_OUT:
            stage1 = outp.tile([128, ngm, HIDDEN], OUT_DT, tag="st1", name="st1")
        for m in range(ngm):
            v = 128 if K_MERGE_OUT else min(128, packtot - 128 * m)
            ps = psum_mm.tile([128, HIDDEN], F32, tag="mm")
            for nt in range(2):
                nc.tensor.matmul(
                    ps[0:v, nt * 512:(nt + 1) * 512],
                    e2t[:, 128 * m:128 * m + v],
                    p2_sb[:, nt * 512:(nt + 1) * 512],
                    start=True, stop=True,
                )
            ot = stage1[:, m, :] if K_MERGE_OUT else outp.tile(
                [128, HIDDEN], OUT_DT, tag="ot")
            if K_COPY_SPLIT:
                nc.vector.tensor_copy(ot[0:v, 0:512], ps[0:v, 0:512])
                nc.scalar.copy(ot[0:v, 512:1024], ps[0:v, 512:1024])
            else:
                nc.vector.tensor_copy(ot[0:v, :], ps[0:v, :])
            if not K_MERGE_OUT:
                r0 = P["t1_base"] + 128 * m
                nc.scalar.dma_start(out[r0:r0 + v, :], ot[0:v, :])
        if K_MERGE_OUT:
            r0 = P["t1_base"]
            dst = out[r0:r0 + ngm * 128, :].rearrange(
                "(g p) h -> p g h", p=128)
            nc.scalar.dma_start(dst, stage1[:])


def _build(P, repeat=1):
    import contextlib
    nc = bacc.Bacc("TRN2", target_bir_lowering=False, debug=False,
                   num_devices=N_CORES, num_swdge_queues=K_QUEUES)
    T = dict(
        tproj=nc.dram_tensor("tproj", [HEAD_V, HIDDEN], OUT_DT, kind="ExternalInput"),
        t0b=nc.dram_tensor("t0b", [2 * CHUNK, 256], T0_DT, kind="ExternalInput"),
        t1=nc.dram_tensor("t1", [N_T1 * CHUNK, 64], F32, kind="ExternalInput"),
        p1=nc.dram_tensor("p1", [128, 2, HIDDEN], T0_DT, kind="ExternalInput"),
        p2=nc.dram_tensor("p2", [64, HIDDEN], F16, kind="ExternalInput"),
        idx=nc.dram_tensor("idx", [128, P["tot16"]], I16, kind="ExternalInput"),
        out=nc.dram_tensor("out", [P["tot_rows"], HIDDEN], OUT_DT, kind="ExternalOutput"),
    )

    with tile.TileContext(nc) as tc:
        with contextlib.ExitStack() as ctx:
            if repeat == 1:
                _emit_body(nc, tc, ctx, P, T)
            else:
                with tc.For_i(0, repeat):
                    _emit_body(nc, tc, ctx, P, T)
    nc.compile()
    return nc


def _weights_maps(head_weight, head_weight_proj, tail_weight_proj_0,
                  tail_weight_0, tail_weight_proj_1, tail_weight_1):
    head_weight = np.asarray(head_weight, np.float32)
    head_weight_proj = np.asarray(head_weight_proj, np.float32)
    # head: fold projection + emb scale into the table
    tproj = np.ascontiguousarray(
        (head_weight[:, :HEAD_V].T @ head_weight_proj.T)
        * np.float32(HIDDEN ** 0.5)).astype(OUT_NP)
    t0b = np.ascontiguousarray(
        np.asarray(tail_weight_0, np.float32).T).astype(T0_NP)
    t1 = np.ascontiguousarray(np.asarray(tail_weight_1, np.float32).T)
    p1 = (np.asarray(tail_weight_proj_0, np.float32).T
          * np.float32(HIDDEN ** 0.5)).astype(T0_NP)
    p1 = np.ascontiguousarray(p1.reshape(2, 128, HIDDEN).transpose(1, 0, 2))
    p2 = np.ascontiguousarray(
        np.asarray(tail_weight_proj_1, np.float32).T
        * np.float32(HIDDEN ** 0.5)).astype(np.float16)
    return dict(tproj=tproj, t0b=t0b, t1=t1, p1=p1, p2=p2)


def _assemble(P, results, x_shape):
    n_tok = int(np.prod(x_shape[:2])) if len(x_shape) > 1 else x_shape[0]
    y = np.zeros((n_tok, HIDDEN), np.float32)
    for c in range(N_CORES):
        o = np.asarray(results[c]["out"], np.float32)
        for s in range(3):  # head, t0c0, t0c1 (linear rows from row_base)
            toks = P["seg_tok"][s][c]
            if len(toks):
                b = P["row_base"][s]
                y[toks] = o[b:b + len(toks)]
        for ci in range(N_T1):
            toks = P["seg_tok"][3 + ci][c]
            if len(toks):
                b = P["t1_base"] + P["pack_off"][ci]
                y[toks] = o[b:b + len(toks)]
    return y.reshape(*x_shape, HIDDEN)


_CACHE = {}


def _get_program(P, repeat=1):
    key = (tuple(P["npad"]), P["packtot"], repeat, K_QUEUES, K_IDX_SPLIT, K_COPY_SPLIT, K_MERGE_OUT)
    if key not in _CACHE:
        _CACHE[key] = _build(P, repeat=repeat)
    return _CACHE[key]


def kernel(x, head_weight, head_weight_proj, tail_weight_proj_0,
           tail_weight_0, tail_weight_proj_1, tail_weight_1):
    x = np.asarray(x)
    P = _plan(x)
    nc = _get_program(P)
    w = _weights_maps(head_weight, head_weight_proj, tail_weight_proj_0,
                      tail_weight_0, tail_weight_proj_1, tail_weight_1)
    in_maps = [dict(w, idx=P["idx"][c]) for c in range(N_CORES)]
    res = run_bass_kernel_spmd(nc, in_maps, core_ids=list(range(N_CORES)))
    return _assemble(P, res.results, x.shape)
